# revision 1
# baseline (speedup 1.0000x reference)
"""GatingAttentionLayerWsa on 8 TRN2 NeuronCores.

Shapes: B=4, S=L=2048, E=512, H=8, D=64.

Sharding: core c = (batch b=c//2, head-group g=c%2 of 4 heads). Each core
computes its batch's Q/K/V projections for its 4 heads, attention with the
row-normalized softmax, and a partial output projection y_part = o_grp @ Wo_grp
(+ bo/2). A ReduceScatter over core pairs {2b, 2b+1} sums the two head-group
partials and leaves disjoint L-halves on each core; the host concatenates.

Math notes:
 - Row mean/var of logits are computed algebraically: mu_l = q_l . k_mean and
   E[z^2]_l = q_l^T (K^T K / S) q_l, so no reduction over the 2048x2048 logits
   is ever done.
 - softmax((z - mu)/std) == softmax(z/std) (the shift cancels), so only
   r = 1/std is applied, folded into q before the QK^T matmul (q~ = q * r).
 - Scores are computed transposed (Z^T [S_part, L_free]) so P^T = exp(Z^T)
   feeds the PV matmul directly (lhsT = [v | 1], moving = P^T). The ones
   column makes row 64 of the output accumulate the softmax row-sums; the
   final per-row normalization is applied to the small [64, L] output.
 - Matmuls run as float32r (inputs rounded once on the producing engine);
   score->exp stays fp32 (PSUM) into the activation.

The build optionally repeats the whole pipeline `reps` times (serialized via
explicit deps on each rep's load DMAs) so per-iteration HW time can be
measured as (t_reps - t_1) / (reps - 1) without device-side profiling.
"""
import sys
from contextlib import ExitStack

import numpy as np

try:
    import concourse.bass as bass  # noqa: F401
except ImportError:  # pragma: no cover
    sys.path.insert(0, "/opt/trn_rl_repo")

import concourse.bacc as bacc
import concourse.mybir as mybir
import concourse.tile as tile
from concourse import masks
from concourse.tile_rust import add_dep_helper
from concourse.bass_utils import run_bass_kernel_spmd

B, S, E, H, D = 4, 2048, 512, 8, 64
L = S
N_CORES = 8
HG = 4            # heads per group
DG = HG * D       # 256 projection cols per group
D1 = D + 1        # head slot width incl ones column
NSC = S // 128    # 16 chunks of 128 along S/L
NEC = E // 128    # 4 chunks of 128 along E
NJ = S // 512     # 4 chunks of 512 along S/L
F32 = mybir.dt.float32
F32R = mybir.dt.float32r
AF = mybir.ActivationFunctionType
GROUPS = [[0, 1], [2, 3], [4, 5], [6, 7]]

_CACHE = {}


def _build(reps=1, stop=99):
    nc = bacc.Bacc("TRN2", target_bir_lowering=False, debug=False,
                   num_devices=N_CORES)
    src_q = nc.dram_tensor("src_q", [S, E], F32, kind="ExternalInput").ap()
    src_k = nc.dram_tensor("src_k", [S, E], F32, kind="ExternalInput").ap()
    src_v = nc.dram_tensor("src_v", [S, E], F32, kind="ExternalInput").ap()
    wq = nc.dram_tensor("wq", [E, DG], F32, kind="ExternalInput").ap()
    wk = nc.dram_tensor("wk", [E, DG], F32, kind="ExternalInput").ap()
    wv = nc.dram_tensor("wv", [E, DG], F32, kind="ExternalInput").ap()
    wo = nc.dram_tensor("wo", [DG, E], F32, kind="ExternalInput").ap()
    bq = nc.dram_tensor("bq", [1, DG], F32, kind="ExternalInput").ap()
    bk = nc.dram_tensor("bk", [1, DG], F32, kind="ExternalInput").ap()
    bv = nc.dram_tensor("bv", [1, DG], F32, kind="ExternalInput").ap()
    bo_half = nc.dram_tensor("bo_half", [1, E], F32, kind="ExternalInput").ap()
    out = nc.dram_tensor("out", [L // 2, E], F32, kind="ExternalOutput").ap()

    with tile.TileContext(nc) as tc, ExitStack() as X:
        sb = X.enter_context(tc.tile_pool(name="sb", bufs=1))
        dram = X.enter_context(tc.tile_pool(name="dram", bufs=1, space="DRAM"))

        # ---- constants (once) ----
        ident = sb.tile([128, 128], F32)
        masks.make_identity(nc, ident[:])
        ones_col = sb.tile([128, 1], F32)
        nc.gpsimd.memset(ones_col[:], 1.0)
        ones64 = sb.tile([64, 1], F32R)
        nc.vector.tensor_copy(ones64[:], ones_col[:64, :])
        ones_row = sb.tile([1, 64], F32)
        nc.gpsimd.memset(ones_row[:], 1.0)
        onesr64 = sb.tile([1, 64], F32R)
        nc.vector.tensor_copy(onesr64[:], ones_row[:])
        ones4 = sb.tile([128, HG], F32)
        nc.gpsimd.memset(ones4[:], 1.0)
        eps128 = sb.tile([128, 1], F32)
        nc.gpsimd.memset(eps128[:], 1e-6)

        prev_tail = [None]  # mybir.Instruction ending the previous rep

        def ld(dst_ap, src_ap):
            i = nc.sync.dma_start(dst_ap, src_ap)
            if prev_tail[0] is not None:
                add_dep_helper(i.ins, prev_tail[0], reason="rep serialization")
            return i

        for rep in range(reps):
            R = f"_r{rep}"

            # ---- biases ----
            def bcast_bias(name, src, width):
                row = sb.tile([1, width], F32, name=f"{name}_row{R}",
                              tag=f"{name}_row")
                ld(row[:], src[:])
                full = sb.tile([128, width], F32, name=f"{name}_b{R}",
                               tag=f"{name}_b")
                nc.gpsimd.partition_broadcast(full[:], row[:])
                return full

            bk_b = bcast_bias("bk", bk, DG)
            bv_b = bcast_bias("bv", bv, DG)
            bo_b = bcast_bias("bo", bo_half, E)
            # bq as per-partition columns (for the transposed q projection)
            bqc = []
            for t in range(2):
                c = sb.tile([128, 1], F32, name=f"bqc{t}{R}", tag=f"bqc{t}")
                ld(c[:], bq[0:1, t * 128:(t + 1) * 128].rearrange("a b -> b a"))
                bqc.append(c)

            wo_t = []
            for e in range(DG // 128):
                t = sb.tile([128, E], F32R, name=f"wo{e}{R}", tag=f"wo{e}")
                ld(t[:], wo[e * 128:(e + 1) * 128, :].bitcast(F32R))
                wo_t.append(t)

            # persistent within the rep (main pool, same tags across reps):
            v_sb = [sb.tile([128, HG * D1], F32R, name=f"vn{i}{R}",
                            tag=f"vn{i}") for i in range(NSC)]
            qT = [sb.tile([64, S], F32R, name=f"qT{h}{R}", tag=f"qT{h}")
                  for h in range(HG)]
            kT = [sb.tile([64, S], F32R, name=f"kT{h}{R}", tag=f"kT{h}")
                  for h in range(HG)]
            ckm = [sb.tile([64, D1], F32R, name=f"ckm{h}{R}", tag=f"ckm{h}")
                   for h in range(HG)]

            with ExitStack() as XB:
                pb = XB.enter_context(tc.tile_pool(name=f"pb{R}", bufs=1))
                # k natural with interleaved ones columns: [k_h | 1] x 4
                k_nat = [pb.tile([128, HG * D1], F32, name=f"kn{i}{R}",
                                 tag=f"kn{i}") for i in range(NSC)]

                # ---- phase 1: load sources, transpose, project ----
                with ExitStack() as XA:
                    pa = XA.enter_context(tc.tile_pool(name=f"pa{R}", bufs=1))
                    ps1 = XA.enter_context(
                        tc.tile_pool(name=f"ps1{R}", bufs=1, space="PSUM"))

                    def load_w(name, src, cols):
                        ts = []
                        for e in range(NEC):
                            t = pa.tile([128, cols], F32R, name=f"{name}{e}{R}",
                                        tag=f"{name}{e}")
                            ld(t[:], src[e * 128:(e + 1) * 128, :].bitcast(F32R))
                            ts.append(t)
                        return ts

                    wq_t = load_w("wq", wq, DG)
                    wk_t = load_w("wk", wk, DG)
                    wv_t = load_w("wv", wv, DG)

                    for tname, src in (("q", src_q), ("k", src_k),
                                       ("v", src_v)):
                        srcT = [pa.tile([128, S], F32R, name=f"sT{tname}{e}{R}",
                                        tag=f"sT{e}") for e in range(NEC)]
                        # load + transpose in groups of 4 chunks
                        for scg in range(0, NSC, 4):
                            nats = []
                            for i in range(4):
                                nat = pa.tile([128, E], F32,
                                              name=f"nat_{tname}{scg + i}{R}",
                                              tag="nat", bufs=6)
                                ld(nat[:],
                                   src[(scg + i) * 128:(scg + i + 1) * 128, :])
                                nats.append(nat)
                            for e in range(NEC):
                                pt = ps1.tile([128, 512], F32,
                                              name=f"pt_{tname}{scg}_{e}{R}",
                                              tag="ptr", bufs=3)
                                for i in range(4):
                                    nc.tensor.transpose(
                                        pt[:, i * 128:(i + 1) * 128],
                                        nats[i][:, e * 128:(e + 1) * 128],
                                        ident[:])
                                nc.vector.tensor_copy(
                                    srcT[e][:, scg * 128:(scg + 4) * 128],
                                    pt[:])
                        if tname == "q":
                            # transposed projection: qT_h directly
                            for t in range(2):
                                for j in range(NJ):
                                    js = slice(j * 512, (j + 1) * 512)
                                    pp = ps1.tile([128, 512], F32,
                                                  name=f"ppq{t}_{j}{R}",
                                                  tag="pp", bufs=3)
                                    for e in range(NEC):
                                        nc.tensor.matmul(
                                            pp[:],
                                            wq_t[e][:, t * 128:(t + 1) * 128],
                                            srcT[e][:, js],
                                            start=(e == 0),
                                            stop=(e == NEC - 1))
                                    nc.vector.tensor_scalar_add(
                                        qT[2 * t][:, js], pp[0:64, :],
                                        bqc[t][0:64, :])
                                    nc.vector.tensor_scalar_add(
                                        qT[2 * t + 1][:, js], pp[64:128, :],
                                        bqc[t][64:128, :])
                        else:
                            w_t = wk_t if tname == "k" else wv_t
                            bias_b = bk_b if tname == "k" else bv_b
                            for sc in range(NSC):
                                pp = ps1.tile([128, DG], F32,
                                              name=f"pp_{tname}{sc}{R}",
                                              tag="pp", bufs=3)
                                for e in range(NEC):
                                    nc.tensor.matmul(
                                        pp[:],
                                        srcT[e][:, sc * 128:(sc + 1) * 128],
                                        w_t[e][:], start=(e == 0),
                                        stop=(e == NEC - 1))
                                dst = k_nat[sc] if tname == "k" else v_sb[sc]
                                d3 = dst[:].rearrange("p (h w) -> p h w", h=HG)
                                nc.vector.tensor_add(
                                    d3[:, :, 0:D],
                                    pp[:].rearrange("p (h w) -> p h w", h=HG),
                                    bias_b[:].rearrange("p (h w) -> p h w",
                                                        h=HG))
                                nc.vector.tensor_copy(
                                    d3[:, :, D:D + 1],
                                    ones4[:].rearrange("p (h w) -> p h w",
                                                       h=HG))

                # ---- phase 2: per head C_aug = K^T [K|1] / S; k transposes --
                with tc.tile_pool(name=f"ps2{R}", bufs=1, space="PSUM") as ps2:
                    for h in range(HG if stop >= 2 else 0):
                        ks = slice(h * D1, h * D1 + D)        # k only
                        ka = slice(h * D1, (h + 1) * D1)      # [k | 1]
                        pc = ps2.tile([64, D1], F32, name=f"pc{h}{R}", tag="pc")
                        for sc in range(NSC):
                            nc.tensor.matmul(pc[:], k_nat[sc][:, ks],
                                             k_nat[sc][:, ka],
                                             start=(sc == 0),
                                             stop=(sc == NSC - 1))
                        nc.vector.tensor_scalar_mul(ckm[h][:], pc[:], 1.0 / S)
                        for scg in range(0, NSC, 4):
                            ptk = ps2.tile([64, 512], F32,
                                           name=f"ptk{h}_{scg}{R}",
                                           tag="ptk", bufs=3)
                            for i in range(4):
                                nc.tensor.transpose(
                                    ptk[:, i * 128:(i + 1) * 128],
                                    k_nat[scg + i][:, ks], ident[:])
                            nc.vector.tensor_copy(
                                kT[h][:, scg * 128:(scg + 4) * 128], ptk[:])

            # late pool: phase 3-6 tiles
            if stop < 3:
                continue
            with ExitStack() as XD:
                pd = XD.enter_context(tc.tile_pool(name=f"pd{R}", bufs=1))
                oT = [pd.tile([128, S], F32R, name=f"oT{t}{R}", tag=f"oT{t}")
                      for t in range(2)]

                # ---- phase 3: row stats -> r = 1/std; q~ = q*r (in place) ----
                with ExitStack() as XC:
                    pcl = XC.enter_context(tc.tile_pool(name=f"pcl{R}", bufs=1))
                    ps3 = XC.enter_context(
                        tc.tile_pool(name=f"ps3{R}", bufs=1, space="PSUM"))
                    e2_4 = pcl.tile([128, S], F32, name=f"e2{R}", tag="e2")
                    mu_4 = pcl.tile([128, S], F32, name=f"mu{R}", tag="mu")
                    for h in range(HG):
                        pu = ps3.tile([65, S], F32, name=f"pu{h}{R}", tag="pu")
                        for j in range(NJ):
                            js = slice(j * 512, (j + 1) * 512)
                            nc.tensor.matmul(pu[:, js], ckm[h][:], qT[h][:, js],
                                             start=True, stop=True)
                        wbuf = pcl.tile([64, S], F32R, name=f"wb{h}{R}",
                                        tag="wb")
                        nc.vector.tensor_mul(wbuf[:], pu[0:64, :],
                                             qT[h][:].bitcast(F32))
                        nc.scalar.activation(mu_4[32 * h:32 * h + 1, :],
                                             pu[64:65, :], AF.Copy)
                        for j in range(NJ):
                            cs = slice(j * 512, (j + 1) * 512)
                            pe2 = ps3.tile([1, 512], F32, name=f"pe2_{h}_{j}{R}",
                                           tag="pe2", bufs=2)
                            nc.tensor.matmul(pe2[:], ones64[:], wbuf[:, cs],
                                             start=True, stop=True)
                            nc.scalar.activation(e2_4[32 * h:32 * h + 1, cs],
                                                  pe2[:], AF.Copy)
                    # var = E2 - mu^2; std = sqrt(var+1e-6)+1e-6; r = 1/std
                    mu2 = pcl.tile([128, S], F32, name=f"mu2{R}", tag="mu2")
                    nc.scalar.activation(mu2[:], mu_4[:], AF.Square)
                    nc.vector.tensor_sub(e2_4[:], e2_4[:], mu2[:])
                    nc.scalar.activation(mu_4[:], e2_4[:], AF.Sqrt,
                                         bias=eps128[:], scale=1.0)
                    nc.vector.tensor_scalar_add(mu_4[:], mu_4[:], 1e-6)
                    for h in range(HG):
                        r_h = pcl.tile([1, S], F32R, name=f"r{h}{R}", tag="rh",
                                       bufs=2)
                        with nc.allow_low_precision(reason="f32r intended"):
                            nc.vector.reciprocal(r_h[:],
                                                 mu_4[32 * h:32 * h + 1, :])
                        for j in range(NJ):
                            js = slice(j * 512, (j + 1) * 512)
                            rbp = ps3.tile([64, 512], F32,
                                           name=f"rb{h}_{j}{R}", tag="pbc",
                                           bufs=2)
                            nc.tensor.matmul(rbp[:], onesr64[:], r_h[:, js],
                                             start=True, stop=True)
                            nc.vector.tensor_mul(qT[h][:, js],
                                                 qT[h][:, js].bitcast(F32),
                                                 rbp[:])

                # ---- phase 4: streaming attention per head ----
                with tc.tile_pool(name=f"ps4{R}", bufs=1, space="PSUM") as ps4:
                    for h in range(HG if stop >= 4 else 0):
                        vs = slice(h * D1, (h + 1) * D1)
                        po = [ps4.tile([65, 1024], F32, name=f"po{h}_{nh}{R}",
                                       tag=f"po{nh}") for nh in range(2)]
                        for sc in range(NSC):
                            ksl = kT[h][:, sc * 128:(sc + 1) * 128]
                            for nh in range(2):
                                pz = ps4.tile([128, 1024], F32,
                                              name=f"pz{h}_{sc}_{nh}{R}",
                                              tag="pz", bufs=2)
                                for j in range(2):
                                    zs = slice(j * 512, (j + 1) * 512)
                                    qs_ = slice(nh * 1024 + j * 512,
                                                nh * 1024 + (j + 1) * 512)
                                    nc.tensor.matmul(pz[:, zs], ksl,
                                                     qT[h][:, qs_],
                                                     start=True, stop=True)
                                psb = pd.tile([128, 1024], F32R,
                                              name=f"psb{h}_{sc}_{nh}{R}",
                                              tag="psb", bufs=3)
                                nc.scalar.activation(psb[:], pz[:], AF.Exp,
                                                     bias=0.0, scale=1.0)
                                for j in range(2):
                                    zs = slice(j * 512, (j + 1) * 512)
                                    nc.tensor.matmul(po[nh][:, zs],
                                                     v_sb[sc][:, vs],
                                                     psb[:, zs],
                                                     start=(sc == 0),
                                                     stop=(sc == NSC - 1))
                        # normalize: o_h = po[:64] * (1 / rowsum)
                        rs_row = pd.tile([1, S], F32R, name=f"rs{h}{R}",
                                         tag="rs")
                        for nh in range(2):
                            nc.vector.tensor_copy(
                                rs_row[:, nh * 1024:(nh + 1) * 1024],
                                po[nh][64:65, :])
                        with nc.allow_low_precision(reason="f32r intended"):
                            nc.vector.reciprocal(rs_row[:],
                                                 rs_row[:].bitcast(F32))
                        invb = pd.tile([64, S], F32, name=f"invb{h}{R}",
                                       tag="invb", bufs=2)
                        for j in range(NJ):
                            js = slice(j * 512, (j + 1) * 512)
                            ibp = ps4.tile([64, 512], F32,
                                           name=f"ib{h}_{j}{R}", tag="pz",
                                           bufs=2)
                            nc.tensor.matmul(ibp[:], onesr64[:], rs_row[:, js],
                                             start=True, stop=True)
                            nc.vector.tensor_copy(invb[:, js], ibp[:])
                        pofs = (h % 2) * 64
                        for nh in range(2):
                            ls = slice(nh * 1024, (nh + 1) * 1024)
                            nc.vector.tensor_mul(
                                oT[h // 2][pofs:pofs + 64, ls],
                                po[nh][0:64, :], invb[:, ls])

                # ---- phase 5: y = o_grp @ Wo_grp + bo/2 ----
                y_dram = dram.tile([L, E], F32, name=f"y_dram{R}", tag="ydram")
                with tc.tile_pool(name=f"ps5{R}", bufs=1, space="PSUM") as ps5:
                    for lc in range(NSC if stop >= 5 else 0):
                        py = ps5.tile([128, E], F32, name=f"py{lc}{R}",
                                      tag="py", bufs=3)
                        for t in range(2):
                            nc.tensor.matmul(
                                py[:], oT[t][:, lc * 128:(lc + 1) * 128],
                                wo_t[t][:], start=(t == 0), stop=(t == 1))
                        y_sb = pd.tile([128, E], F32, name=f"y{lc}{R}",
                                       tag="y", bufs=3)
                        nc.vector.tensor_add(y_sb[:], py[:], bo_b[:])
                        nc.sync.dma_start(y_dram[lc * 128:(lc + 1) * 128, :],
                                          y_sb[:])

                # ---- phase 6: ReduceScatter over the core pair, store ----
                rs_out = dram.tile([L // 2, E], F32, name=f"rs_out{R}",
                                   tag="rsout")
                nc.gpsimd.collective_compute(
                    "ReduceScatter", mybir.AluOpType.add, replica_groups=GROUPS,
                    ins=[y_dram[:]], outs=[rs_out[:]])
                tail = nc.sync.dma_start(out[:], rs_out[:])
                prev_tail[0] = tail.ins
    nc.compile()
    return nc


def _get_nc(reps=1):
    key = f"nc{reps}"
    if key not in _CACHE:
        _CACHE[key] = _build(reps)
    return _CACHE[key]


def _in_maps(query, key, value, Wq, bq, Wk, bk, Wv, bv, Wo, bo):
    maps = []
    for c in range(N_CORES):
        b, g = c // 2, c % 2
        cs = slice(g * DG, (g + 1) * DG)
        maps.append({
            "src_q": np.ascontiguousarray(query[b]),
            "src_k": np.ascontiguousarray(key[b]),
            "src_v": np.ascontiguousarray(value[b]),
            "wq": np.ascontiguousarray(Wq[:, cs]),
            "wk": np.ascontiguousarray(Wk[:, cs]),
            "wv": np.ascontiguousarray(Wv[:, cs]),
            "wo": np.ascontiguousarray(Wo[cs, :]),
            "bq": np.ascontiguousarray(bq[cs]).reshape(1, DG),
            "bk": np.ascontiguousarray(bk[cs]).reshape(1, DG),
            "bv": np.ascontiguousarray(bv[cs]).reshape(1, DG),
            "bo_half": (0.5 * np.asarray(bo)).reshape(1, E),
        })
    return maps


def kernel(**inputs):
    inputs = {k: np.asarray(v, dtype=np.float32) for k, v in inputs.items()}
    nc = _get_nc()
    maps = _in_maps(**inputs)
    res = run_bass_kernel_spmd(nc, maps, list(range(N_CORES)))
    out = np.empty((B, L, E), dtype=np.float32)
    for b in range(B):
        out[b, :L // 2] = res.results[2 * b]["out"]
        out[b, L // 2:] = res.results[2 * b + 1]["out"]
    _CACHE["last_maps"] = maps
    return out


def _timed_fn(reps):
    """Jitted sharded single-call executable with device-resident buffers."""
    import jax
    from jax.sharding import Mesh, PartitionSpec, NamedSharding
    from jax.experimental.shard_map import shard_map
    from concourse.bass2jax import (_bass_exec_p, partition_id_tensor,
                                    install_neuronx_cc_hook)

    nc = _get_nc(reps)
    install_neuronx_cc_hook()
    in_names, out_names, out_avals = [], [], []
    for alloc in nc.m.functions[0].allocations:
        if not isinstance(alloc, mybir.MemoryLocationSet):
            continue
        name = alloc.memorylocations[0].name
        if alloc.kind == "ExternalInput":
            if name != "partition_id":
                in_names.append(name)
        elif alloc.kind == "ExternalOutput":
            out_names.append(name)
            out_avals.append(jax.core.ShapedArray(
                tuple(alloc.tensor_shape), mybir.dt.np(alloc.dtype)))
    n_params, n_outs = len(in_names), len(out_names)
    all_in = in_names + out_names + ["partition_id"]

    def _body(*args):
        outs = _bass_exec_p.bind(
            *args, partition_id_tensor(),
            out_avals=tuple(out_avals), in_names=tuple(all_in),
            out_names=tuple(out_names), lowering_input_output_aliases=(),
            sim_require_finite=True, sim_require_nnan=True, nc=nc)
        return tuple(outs)

    devices = jax.devices()[:N_CORES]
    mesh = Mesh(np.asarray(devices), ("core",))
    sh = NamedSharding(mesh, PartitionSpec("core"))
    fn = jax.jit(
        shard_map(_body, mesh=mesh,
                  in_specs=(PartitionSpec("core"),) * (n_params + n_outs),
                  out_specs=(PartitionSpec("core"),) * n_outs,
                  check_rep=False),
        keep_unused=True)
    maps = _CACHE["last_maps"]
    darg = [jax.device_put(
                np.concatenate([np.asarray(maps[c][n]) for c in range(N_CORES)],
                               axis=0), sh) for n in in_names]
    darg += [jax.device_put(
                np.zeros((N_CORES * a.shape[0], *a.shape[1:]), a.dtype), sh)
             for a in out_avals]

    def call():
        import jax as _j
        return _j.block_until_ready(fn(*darg))

    return call


def measure_exec_time_ns(reps=8, trials=10):
    """Per-iteration HW time via in-NEFF repetition delta."""
    import time
    call1 = _timed_fn(1)
    callN = _timed_fn(reps)
    call1(); callN()  # warm both executables

    def best(call):
        b = float("inf")
        for _ in range(trials):
            t0 = time.perf_counter()
            call()
            b = min(b, time.perf_counter() - t0)
        return b

    t1, tN = best(call1), best(callN)
    return int((tN - t1) / (reps - 1) * 1e9)


if __name__ == "__main__":
    nc = _get_nc()
    print("built + compiled ok")

